# revision 13
# baseline (speedup 1.0000x reference)
"""AugmentedGeneEmbedding kernel for 8 TRN2 NeuronCores (Bass/Tile).

Math (per token t with gene g = idx[t]):
    id_vec  = id_table[g]                                  # [128]
    e       = gene_idx_to_esm_idx[g]
    valid   = (g < N_GENES) & (0 < e < V_ESM)
    seq     = valid ? esm_table[e] @ Wp + bp : 0           # [256]
    h       = concat([id_vec, tanh(gate) * seq])           # [384]
    y       = gelu(h @ W1 + b1) @ W2 + b2                  # [128]

Kernel restructuring (algebraically identical):
    z = h @ W1 + b1
      = id_vec @ W1[:128]                    (id path)
      + valid * (esm_row @ (Wp @ tanh(g)W1[128:]))   (esm path, Wp@W1_bot folded on device)
      + valid * (bp @ tanh(g)W1[128:])       (rank-1 mask term, K=1 matmul)
      + b1
    y = gelu(z) @ W2 + b2

Everything is computed TRANSPOSED (feature dim on SBUF partitions, tokens on
the free dim) so that transpose-mode dma_gather delivers gathered table rows
directly in matmul operand orientation.  Sharding: data-parallel over tokens
(8192 tokens/core), tables replicated.
"""

import numpy as np
import ml_dtypes

N_CORES = 8
B, K = 32, 2048
N_GENES, ID_DIM, ESM_DIM, PROJ, V_ESM = 20000, 128, 1280, 256, 30000
NTOK_TOTAL = B * K
NTOK = NTOK_TOTAL // N_CORES  # 8192 tokens per core

GT = 512   # tokens per dma_gather tile (descriptor-ring limit: num_idxs*elem_bytes/4KB + 2 descs per engine, keep <= ~512 in flight)
MC = 512   # tokens per matmul chunk (max moving free dim)

BF16 = ml_dtypes.bfloat16

_BUILD_CACHE = {}


def build_nc(n_tok):
    """Build + compile the per-core Bass program (SPMD: same program, 8 cores)."""
    import concourse.bacc as bacc
    import concourse.mybir as mybir
    import concourse.tile as tile
    from contextlib import ExitStack

    fp32 = mybir.dt.float32
    bf16 = mybir.dt.bfloat16
    i16 = mybir.dt.int16
    AF = mybir.ActivationFunctionType

    assert n_tok % GT == 0
    n_gt = n_tok // GT

    nc = bacc.Bacc("TRN2", target_bir_lowering=False, num_swdge_queues=4)

    eidx_d = nc.declare_dram_parameter("eidx16", [128, n_tok // 16], i16, isOutput=False)
    idid_d = nc.declare_dram_parameter("idid16", [128, n_tok // 16], i16, isOutput=False)
    mask_d = nc.declare_dram_parameter("maskbf", [1, n_tok], bf16, isOutput=False)
    esm_d = nc.declare_dram_parameter("esmbf", [V_ESM + 1, ESM_DIM], bf16, isOutput=False)
    id_d = nc.declare_dram_parameter("idbf", [N_GENES, ID_DIM], bf16, isOutput=False)
    wp_d = nc.declare_dram_parameter("wpbf", [ESM_DIM, PROJ], bf16, isOutput=False)
    w1_d = nc.declare_dram_parameter("w1bf", [ID_DIM + PROJ, PROJ], bf16, isOutput=False)
    w2_d = nc.declare_dram_parameter("w2bf", [PROJ, ID_DIM], bf16, isOutput=False)
    bp_d = nc.declare_dram_parameter("bpw", [128, 2], bf16, isOutput=False)
    b1_d = nc.declare_dram_parameter("b1w", [128, 2], fp32, isOutput=False)
    b2_d = nc.declare_dram_parameter("b2row", [1, 128], fp32, isOutput=False)
    gate_d = nc.declare_dram_parameter("gatef", [1, 1], fp32, isOutput=False)
    out_d = nc.declare_dram_parameter("out", [n_tok, 128], bf16, isOutput=True)

    with tile.TileContext(nc) as tc, ExitStack() as ctx:
        const = ctx.enter_context(tc.tile_pool(name="const", bufs=1))
        gpool = ctx.enter_context(tc.tile_pool(name="gather", bufs=4))
        apool = ctx.enter_context(tc.tile_pool(name="act", bufs=4))
        ypool = ctx.enter_context(tc.tile_pool(name="yout", bufs=3))
        zps = ctx.enter_context(tc.tile_pool(name="zps", bufs=3, space="PSUM"))
        yps = ctx.enter_context(tc.tile_pool(name="yps", bufs=3, space="PSUM"))

        # ---------- constants ----------
        # Dummy 16-idx gather issued first so the auto-inserted GPSIMD library
        # swap (a barrier on the gpsimd queue) happens during the NEFF preamble
        # instead of after all const DMAs.
        dummy_idx = const.tile([128, 1], i16)
        nc.vector.memset(dummy_idx[:], 0)
        dummy_out = const.tile([128, 1, 128], bf16)
        nc.gpsimd.dma_gather(dummy_out[:], esm_d[:, 0:128], dummy_idx[:], 16, 16, 128,
                             elem_step=ESM_DIM)

        # Index tiles load on the scalar HWDGE queue so the gathers (gpsimd)
        # can start immediately; fold-critical weight loads go on sync.
        eidx_sb = const.tile([128, n_tok // 16], i16)
        nc.scalar.dma_start(eidx_sb[:], eidx_d[:])
        idid_sb = const.tile([128, n_tok // 16], i16)
        nc.scalar.dma_start(idid_sb[:], idid_d[:])

        # Software-pipelined gathers: issue PREFETCH tiles up front (right
        # after the index loads, before the weight fold) so the Q7 descriptor
        # generation and gather DMAs run under the whole prologue.
        PREFETCH = 4
        pending = []

        def issue_gather(g):
            ic = g * (GT // 16)
            gtile = gpool.tile([128, 10, GT], bf16, tag="G", name=f"G{g}")
            nc.gpsimd.dma_gather(gtile[:], esm_d[:],
                                 eidx_sb[:, ic:ic + GT // 16], GT, GT, ESM_DIM,
                                 transpose=True, queue_num=g % 4)
            itile = gpool.tile([128, 1, GT], bf16, tag="I", name=f"I{g}")
            nc.gpsimd.dma_gather(itile[:], id_d[:],
                                 idid_sb[:, ic:ic + GT // 16], GT, GT, ID_DIM,
                                 transpose=True, queue_num=(g + 1) % 4)
            pending.append((gtile, itile))

        for g in range(min(PREFETCH, n_gt)):
            issue_gather(g)

        # Fold-critical weight loads come after the gather issuance in program
        # order so the auto-inserted GPSIMD library swap does not inherit
        # their DMA-completion waits (it stalls the whole gather queue).
        wpT0 = const.tile([128, ESM_DIM], bf16)
        nc.sync.dma_start(wpT0[:], wp_d[:, 0:128], transpose=True)
        wpT1 = const.tile([128, ESM_DIM], bf16)
        nc.sync.dma_start(wpT1[:], wp_d[:, 128:256], transpose=True)
        w1b_sb = const.tile([128, 2, PROJ], bf16)
        nc.sync.dma_start(w1b_sb[:], w1_d[128:384, :].rearrange("(c p) f -> p c f", p=128))
        gate_sb = const.tile([1, 1], fp32)
        nc.sync.dma_start(gate_sb[:], gate_d[:])
        w1t_sb = const.tile([128, PROJ], bf16)
        nc.sync.dma_start(w1t_sb[:], w1_d[0:128, :])
        mask_sb = const.tile([1, n_tok], bf16)
        nc.scalar.dma_start(mask_sb[:], mask_d[:])
        w2_sb = const.tile([128, 2, 128], bf16)
        nc.scalar.dma_start(w2_sb[:], w2_d[:].rearrange("(c p) f -> p c f", p=128))
        bp_sb = const.tile([128, 2], bf16)
        nc.sync.dma_start(bp_sb[:], bp_d[:])
        b1_sb = const.tile([128, 2], fp32)
        nc.scalar.dma_start(b1_sb[:], b1_d[:])
        b2_sb = const.tile([1, 128], fp32)
        nc.scalar.dma_start(b2_sb[:], b2_d[:])

        ones1 = const.tile([1, 128], fp32)
        nc.vector.memset(ones1[:], 1.0)

        # ---------- one-time weight folding ----------
        tg_sb = const.tile([128, 1], fp32)         # tanh(gate) on every partition
        b2b_sb = const.tile([128, 128], fp32)      # b2 broadcast across partitions
        wc_sb = const.tile([128, 10, PROJ], bf16)  # Wc = tanh(g) * (Wp @ W1_bot)
        cb_sb = const.tile([1, PROJ], bf16)        # cb = tanh(g) * (bp @ W1_bot)
        with tc.tile_pool(name="foldps", bufs=2, space="PSUM") as fps:
            gb_ps = fps.tile([128, 1], fp32, tag="fold")
            nc.tensor.matmul(gb_ps[:], ones1[:], gate_sb[:], start=True, stop=True)
            nc.scalar.activation(tg_sb[:], gb_ps[:], AF.Tanh)

            b2b_ps = fps.tile([128, 128], fp32, tag="fold")
            nc.tensor.matmul(b2b_ps[:], ones1[:], b2_sb[:], start=True, stop=True)
            nc.vector.tensor_copy(b2b_sb[:], b2b_ps[:])

            for c in range(10):
                wc_ps = fps.tile([128, PROJ], fp32, tag="fold")
                nc.tensor.matmul(wc_ps[:], wpT0[:, c * 128:(c + 1) * 128],
                                 w1b_sb[:, 0, :], start=True, stop=False)
                nc.tensor.matmul(wc_ps[:], wpT1[:, c * 128:(c + 1) * 128],
                                 w1b_sb[:, 1, :], start=False, stop=True)
                nc.scalar.activation(wc_sb[:, c, :], wc_ps[:], AF.Copy,
                                     scale=tg_sb[:, 0:1])

            cb_ps = fps.tile([1, PROJ], fp32, tag="fold")
            nc.tensor.matmul(cb_ps[:], bp_sb[:, 0:1], w1b_sb[:, 0, :], start=True, stop=False)
            nc.tensor.matmul(cb_ps[:], bp_sb[:, 1:2], w1b_sb[:, 1, :], start=False, stop=True)
            nc.scalar.activation(cb_sb[:], cb_ps[:], AF.Copy, scale=tg_sb[0:1, 0:1])

        # ---------- main token pipeline ----------
        for g in range(n_gt):
            gtile, itile = pending.pop(0)
            if g + PREFETCH < n_gt:
                issue_gather(g + PREFETCH)
            for m in range(GT // MC):
                t0 = m * MC
                goff = g * GT + t0
                a_tiles = []
                for h in range(2):
                    hs = slice(h * 128, (h + 1) * 128)
                    zp = zps.tile([128, MC], fp32, tag="z")
                    nc.tensor.matmul(zp[:], w1t_sb[:, hs], itile[:, 0, t0:t0 + MC],
                                     start=True, stop=False)
                    for c in range(10):
                        nc.tensor.matmul(zp[:], wc_sb[:, c, hs],
                                         gtile[:, c, t0:t0 + MC],
                                         start=False, stop=False)
                    nc.tensor.matmul(zp[:], cb_sb[0:1, hs],
                                     mask_sb[0:1, goff:goff + MC],
                                     start=False, stop=True)
                    at = apool.tile([128, MC], bf16, tag="a")
                    nc.scalar.activation(at[:], zp[:], AF.Gelu, bias=b1_sb[:, h:h + 1])
                    a_tiles.append(at)
                ysb = ypool.tile([128, MC // 128, 128], bf16, tag="y")
                for q in range(MC // 128):
                    qs = slice(q * 128, (q + 1) * 128)
                    yp = yps.tile([128, 128], fp32, tag="yp")
                    nc.tensor.matmul(yp[:], a_tiles[0][:, qs], w2_sb[:, 0, :],
                                     start=True, stop=False)
                    nc.tensor.matmul(yp[:], a_tiles[1][:, qs], w2_sb[:, 1, :],
                                     start=False, stop=True)
                    nc.vector.tensor_add(ysb[:, q, :], yp[:], b2b_sb[:])
                nc.sync.dma_start(
                    out_d[goff:goff + MC, :].rearrange("(q p) f -> p q f", p=128),
                    ysb[:])

    nc.compile()
    return nc


def _wrap16(a16):
    """int16 [n] -> [128, n//16]: logical index i at [i % 16 (+16k), i // 16]."""
    w = a16.reshape(-1, 16).T
    return np.tile(w, (8, 1)).copy()


def prepare_host(idx, gene_idx_to_esm_idx, id_table, esm_table, Wp, bp, gate,
                 W1, b1, W2, b2, n_cores=N_CORES):
    """Index prep + dtype/layout marshalling. Returns (shared, per_core) maps."""
    idx_flat = np.asarray(idx).reshape(-1).astype(np.int64)
    gmap = np.asarray(gene_idx_to_esm_idx).astype(np.int64)
    eidx = gmap[np.clip(idx_flat, 0, N_GENES - 1)]
    valid = (idx_flat >= 0) & (idx_flat < N_GENES) & (eidx > 0) & (eidx < V_ESM)
    esm_gidx = np.where(valid, eidx, V_ESM).astype(np.int16)  # row V_ESM is zero pad
    id_gidx = np.clip(idx_flat, 0, N_GENES - 1).astype(np.int16)
    mask = valid.astype(BF16)

    shared = {
        "esmbf": np.concatenate(
            [np.asarray(esm_table).astype(BF16), np.zeros((1, ESM_DIM), BF16)], axis=0),
        "idbf": np.asarray(id_table).astype(BF16),
        "wpbf": np.asarray(Wp).astype(BF16),
        "w1bf": np.asarray(W1).astype(BF16),
        "w2bf": np.asarray(W2).astype(BF16),
        "bpw": np.asarray(bp).astype(BF16).reshape(2, 128).T.copy(),
        "b1w": np.asarray(b1).astype(np.float32).reshape(2, 128).T.copy(),
        "b2row": np.asarray(b2).astype(np.float32).reshape(1, 128).copy(),
        "gatef": np.asarray(gate).astype(np.float32).reshape(1, 1).copy(),
    }
    n_tok = idx_flat.shape[0] // n_cores
    per_core = []
    for c in range(n_cores):
        s = slice(c * n_tok, (c + 1) * n_tok)
        per_core.append({
            "eidx16": _wrap16(esm_gidx[s]),
            "idid16": _wrap16(id_gidx[s]),
            "maskbf": mask[s].reshape(1, -1).copy(),
        })
    return shared, per_core


def kernel(idx, gene_idx_to_esm_idx, id_table, esm_table, Wp, bp, gate,
           W1, b1, W2, b2, _trace=False, **_run_kwargs):
    from concourse.bass_utils import run_bass_kernel_spmd

    shared, per_core = prepare_host(idx, gene_idx_to_esm_idx, id_table, esm_table,
                                    Wp, bp, gate, W1, b1, W2, b2)
    if NTOK not in _BUILD_CACHE:
        _BUILD_CACHE[NTOK] = build_nc(NTOK)
    nc = _BUILD_CACHE[NTOK]

    in_maps = [dict(shared, **pc) for pc in per_core]
    res = run_bass_kernel_spmd(nc, in_maps, list(range(N_CORES)), trace=_trace,
                               **_run_kwargs)
    out = np.concatenate([np.asarray(res.results[c]["out"]) for c in range(N_CORES)],
                         axis=0)
    out = out.reshape(B, K, ID_DIM).astype(np.float32)
    if _trace:
        return out, res
    return out
